# revision 1
# baseline (speedup 1.0000x reference)
"""Trainium2 Bass kernel for causal top-8 sparse attention (nn_DGN7).

Math (see reference):
  A    = top-8 strictly-causal neighbours of each row by x.x^T similarity
  attn = softmax over the selected scores, score = (x Wq^T)(x Wk^T)^T/sqrt(32)
  out  = gelu_exact((mix*x + (1-mix)*attn@x) * gain + bias) * (softplus+0.01)

Sharding: 8 cores; core i handles batch i//4 and, for every prefix level
l=1..8, the 128-row tile g = 4*(l-1) + (i%4).  Every core runs an identical
static program over strips of width 512*l (l=1..8); total causal area is
exactly balanced across cores.

Numerics:
  - similarity on PE via fp16 hi/lo split (3 matmuls: h.h' + (h/2^11).(l'*2^11)
    + (l*2^11).(h'/2^11)) -> ~fp32-level error (HW measured rel ~4.5e-7).
  - top-8 via DVE InstMax + match_replace (exact, first-match tie-break).
  - q/k/score/msg matmuls in fp16 (errors ~2^-11, well within tolerance).
  - softmax without max-shift (scores bounded; exp biased by -4 for fp16
    range); Z via ACT accum_out; normalisation after the msg matmul.
  - selection mask applied additively to the scores (0 / -3e38), -3e38 chosen
    to stay finite in bf16.
  - gain/bias/mix folded host-side: xrm = mix*x*gain + bias, gainb=(1-mix)*gain.
Host does layout prep (transposes/fp16 piece casts) + the degenerate t=0 rows.
"""
import math
import numpy as np
import ml_dtypes

import concourse.bass as bass
import concourse.mybir as mybir
from concourse import bacc
from concourse.tile import TileContext
from concourse.bass_utils import run_bass_kernel_spmd

B, T, D = 2, 4096, 1024
DH = 32
P = 128
PANEL = 512
NLEV = 8
NCHUNK = D // P          # 8
NCORES = 8
FMIN = float(np.finfo(np.float32).min)
MASKTHR = -1e38
MASKVAL = -3.0e38        # finite in bf16 (FMIN would round to -inf)
SPLIT = 2048.0           # 2^11 lo-piece scale
ESHIFT = -4.0            # exp input bias (fp16 range safety)

f32 = mybir.dt.float32
f16 = mybir.dt.float16
bf16 = mybir.dt.bfloat16

_prog_cache = {}


def _build_program(act_fn=None):
    if act_fn is None:
        act_fn = mybir.ActivationFunctionType.Gelu
    nc = bacc.Bacc(trn_type="TRN2")

    # ---------------- DRAM I/O ----------------
    d_xbT_h = nc.dram_tensor("xbT_h", [D, T], f16, kind="ExternalInput")
    d_xbT_ls = nc.dram_tensor("xbT_ls", [D, T], f16, kind="ExternalInput")
    d_xb_h = nc.dram_tensor("xb_h", [T, D], f16, kind="ExternalInput")
    d_xrT_h = nc.dram_tensor("xrT_h", [NLEV, NCHUNK, P, P], f16, kind="ExternalInput")
    d_xrT_ls = nc.dram_tensor("xrT_ls", [NLEV, NCHUNK, P, P], f16, kind="ExternalInput")
    d_xrT_hs = nc.dram_tensor("xrT_hs", [NLEV, NCHUNK, P, P], f16, kind="ExternalInput")
    d_xrm = nc.dram_tensor("xrm", [NLEV, P, D], f16, kind="ExternalInput")
    d_maskdiag = nc.dram_tensor("maskdiag", [P, PANEL], bf16, kind="ExternalInput")
    d_wqT = nc.dram_tensor("wqT", [D, DH], f16, kind="ExternalInput")
    d_wkT = nc.dram_tensor("wkT", [D, DH], f16, kind="ExternalInput")
    d_ident16 = nc.dram_tensor("ident16", [P, P], f16, kind="ExternalInput")
    d_identbf = nc.dram_tensor("identbf", [P, P], bf16, kind="ExternalInput")
    d_grow = nc.dram_tensor("grow", [1, D], f32, kind="ExternalInput")
    d_zerocol = nc.dram_tensor("zerocol", [P, 1], f32, kind="ExternalInput")
    d_scalecol = nc.dram_tensor("scalecol", [P, 1], f32, kind="ExternalInput")
    d_eshift = nc.dram_tensor("eshift", [P, 1], f32, kind="ExternalInput")
    d_onesrow = nc.dram_tensor("onesrow", [1, P], f32, kind="ExternalInput")
    d_out = nc.dram_tensor("out", [NLEV, P, D], f32, kind="ExternalOutput")

    with TileContext(nc) as tc:
        with tc.tile_pool(name="const", bufs=1) as cpool, \
             tc.tile_pool(name="strips", bufs=1) as spool, \
             tc.tile_pool(name="big", bufs=1) as bpool, \
             tc.tile_pool(name="panels", bufs=2) as ppool, \
             tc.tile_pool(name="attn", bufs=1) as apool, \
             tc.tile_pool(name="work", bufs=2) as wpool, \
             tc.tile_pool(name="work1", bufs=1) as w1pool, \
             tc.tile_pool(name="panels1", bufs=1) as p1pool, \
             tc.tile_pool(name="simP", bufs=2, space="PSUM") as simP, \
             tc.tile_pool(name="miscP", bufs=2, space="PSUM") as miscP, \
             tc.tile_pool(name="tranP", bufs=2, space="PSUM") as tranP, \
             tc.tile_pool(name="msgP", bufs=2, space="PSUM") as msgP:

            # ---------------- constants / resident tensors ----------------
            wq_sb = cpool.tile([P, NCHUNK, DH], f16)
            wk_sb = cpool.tile([P, NCHUNK, DH], f16)
            for c in range(NCHUNK):
                nc.sync.dma_start(wq_sb[:, c], d_wqT[P * c:P * (c + 1), :])
                nc.sync.dma_start(wk_sb[:, c], d_wkT[P * c:P * (c + 1), :])
            ident16 = cpool.tile([P, P], f16)
            nc.sync.dma_start(ident16, d_ident16.ap())
            identbf = cpool.tile([P, P], bf16)
            nc.sync.dma_start(identbf, d_identbf.ap())
            maskdiag = cpool.tile([P, PANEL], bf16)
            nc.sync.dma_start(maskdiag, d_maskdiag.ap())
            zerocol = cpool.tile([P, 1], f32)
            nc.sync.dma_start(zerocol, d_zerocol.ap())
            scalecol = cpool.tile([P, 1], f32)
            nc.sync.dma_start(scalecol, d_scalecol.ap())
            eshiftcol = cpool.tile([P, 1], f32)
            nc.sync.dma_start(eshiftcol, d_eshift.ap())
            onesrow = cpool.tile([1, P], f32)
            nc.sync.dma_start(onesrow, d_onesrow.ap())
            growrow = w1pool.tile([1, D], f32, tag="g", name="growrow")
            nc.sync.dma_start(growrow, d_grow.ap())

            # (1-mix)*gain broadcast to all partitions via K=1 matmuls
            gainb = cpool.tile([P, D], f16)
            for k in range(2):
                gps = msgP.tile([P, PANEL], f32, tag="msg")
                nc.tensor.matmul(gps, onesrow,
                                 growrow[:, PANEL * k:PANEL * (k + 1)],
                                 start=True, stop=True)
                nc.scalar.copy(gainb[:, PANEL * k:PANEL * (k + 1)], gps)

            # own-row lhsT pieces, resident (6 MB), per-level tiles so the
            # first sim matmuls only wait for their own level's DMAs
            xr_h, xr_ls, xr_hs = [], [], []
            for l in range(NLEV):
                th = bpool.tile([P, NCHUNK, P], f16, tag=f"xrh{l}", name=f"xrh{l}")
                tl = bpool.tile([P, NCHUNK, P], f16, tag=f"xrl{l}", name=f"xrl{l}")
                ts_ = bpool.tile([P, NCHUNK, P], f16, tag=f"xrs{l}", name=f"xrs{l}")
                nc.gpsimd.dma_start(th, d_xrT_h[l].rearrange("c d t -> d c t"))
                nc.gpsimd.dma_start(tl, d_xrT_ls[l].rearrange("c d t -> d c t"))
                nc.gpsimd.dma_start(ts_, d_xrT_hs[l].rearrange("c d t -> d c t"))
                xr_h.append(th); xr_ls.append(tl); xr_hs.append(ts_)

            kT_sb = cpool.tile([DH, T], f16)        # k^T, filled per panel
            strip = [spool.tile([P, PANEL * (l + 1)], f32, tag=f"strip{l}",
                                name=f"strip{l}")
                     for l in range(NLEV)]
            qT = [cpool.tile([DH, P], f16, tag=f"qT{l}", name=f"qT{l}")
                  for l in range(NLEV)]

            def emit_sim_tile(l, p, ph, pls, phs):
                ps = simP.tile([P, PANEL], f32, tag="sim")
                li = l - 1
                last = (p == l - 1)
                mms = []
                for c in range(NCHUNK):
                    mms.append((xr_h[li][:, c], ph[:, c]))
                for c in range(NCHUNK):
                    mms.append((xr_hs[li][:, c], pls[:, c]))
                for c in range(NCHUNK):
                    mms.append((xr_ls[li][:, c], phs[:, c]))
                if last:
                    mms.append((identbf, maskdiag))
                for i, (lhsT, rhs) in enumerate(mms):
                    nc.tensor.matmul(ps, lhsT, rhs,
                                     start=(i == 0), stop=(i == len(mms) - 1))
                nc.scalar.copy(strip[li][:, PANEL * p:PANEL * (p + 1)], ps)

            def emit_level_b(l):
                """selection + score/exp/attn^T + msg + out for level l"""
                li = l - 1
                st = strip[li]
                # --- q^T for this level ---
                qps = miscP.tile([P, PANEL], f32, tag="misc")
                for c in range(NCHUNK):
                    nc.tensor.matmul(qps[:DH, :P], wq_sb[:, c], xr_h[li][:, c],
                                     start=(c == 0), stop=(c == NCHUNK - 1))
                nc.scalar.copy(qT[li], qps[:DH, :P])
                # --- selection ---
                top8 = w1pool.tile([P, 8], f32, tag="top8")
                nc.vector.max(out=top8, in_=st)
                nc.vector.match_replace(out=st, in_to_replace=top8,
                                        in_values=st, imm_value=FMIN)
                # additive mask in place: -0.0 where selected/invalid, -3e38 else
                nc.vector.tensor_scalar(st, st, MASKTHR, scalar2=MASKVAL,
                                        op0=mybir.AluOpType.is_gt,
                                        op1=mybir.AluOpType.mult)
                # --- attn^T strip + Z ---
                attnT = apool.tile([P, 4 * NLEV, P], f16, tag="attnT")
                zcols = w1pool.tile([P, NLEV], f32, tag="zcols")
                for c in range(l):
                    sps = miscP.tile([P, PANEL], f32, tag="misc")
                    nc.tensor.matmul(sps, qT[li],
                                     kT_sb[:, PANEL * c:PANEL * (c + 1)],
                                     start=True, stop=(c != l - 1))
                    if c == l - 1:
                        nc.tensor.matmul(sps, identbf, maskdiag,
                                         start=False, stop=True)
                    nc.vector.tensor_add(sps, sps,
                                         st[:, PANEL * c:PANEL * (c + 1)])
                    au = wpool.tile([P, PANEL], f16, tag="au")
                    nc.scalar.activation(au, sps,
                                         mybir.ActivationFunctionType.Exp,
                                         bias=eshiftcol, scale=1.0,
                                         accum_out=zcols[:, c:c + 1])
                    tp = tranP.tile([P, PANEL], f16, tag="tran")
                    for q in range(4):
                        nc.tensor.matmul(tp[:, P * q:P * (q + 1)],
                                         au[:, P * q:P * (q + 1)], ident16,
                                         is_transpose=True,
                                         start=(q == 0), stop=(q == 3))
                    nc.scalar.copy(
                        attnT[:, 4 * c:4 * (c + 1)].rearrange("p b t -> p (b t)"),
                        tp)
                # --- Z -> 1/Z (per-partition column) ---
                zsum = w1pool.tile([P, 1], f32, tag="zsum")
                nc.vector.tensor_reduce(
                    out=zsum, in_=zcols[:, :l], op=mybir.AluOpType.add,
                    axis=mybir.AxisListType.X)
                nc.vector.tensor_scalar_max(zsum, zsum, 1e-30)
                zrec = w1pool.tile([P, 1], f32, tag="zrec")
                nc.vector.reciprocal(zrec, zsum)
                # --- msg (natural layout): lhsT = attnT block (stationary) ---
                mp0 = msgP.tile([P, PANEL], f32, tag="msg")
                mp1 = msgP.tile([P, PANEL], f32, tag="msg")
                mps = [mp0, mp1]
                nblk = 4 * l
                for p2 in range(l):
                    xbh = ppool.tile([P, 4, D], f16, tag="xbh")
                    for sb in range(4):
                        nc.scalar.dma_start(
                            xbh[:, sb],
                            d_xb_h[PANEL * p2 + P * sb:PANEL * p2 + P * (sb + 1), :])
                    for sb in range(4):
                        blk = 4 * p2 + sb
                        for k in range(2):
                            nc.tensor.matmul(
                                mps[k], attnT[:, blk],
                                xbh[:, sb, PANEL * k:PANEL * (k + 1)],
                                start=(blk == 0), stop=(blk == nblk - 1))
                # --- out stage (t-major) ---
                xrm = w1pool.tile([P, D], f16, tag="xrm")
                nc.gpsimd.dma_start(xrm, d_xrm[li])
                for k in range(2):
                    sl = slice(PANEL * k, PANEL * (k + 1))
                    gh = w1pool.tile([P, PANEL], f32, tag="g", name=f"g{k}")
                    nc.vector.tensor_scalar_mul(gh, mps[k], zrec)
                    nc.vector.tensor_mul(gh, gh, gainb[:, sl])
                    nc.vector.tensor_add(gh, gh, xrm[:, sl])
                    nc.scalar.activation(gh, gh, act_fn, bias=zerocol, scale=1.0)
                    nc.vector.tensor_scalar_mul(gh, gh, scalecol)
                    nc.sync.dma_start(d_out[li][:, sl], gh)

            # ---------------- main pipeline ----------------
            for p in range(NLEV):
                ph = ppool.tile([P, NCHUNK, PANEL], f16, tag="ph")
                pls = ppool.tile([P, NCHUNK, PANEL], f16, tag="pls")
                phs = p1pool.tile([P, NCHUNK, PANEL], f16, tag="phs")
                for c in range(NCHUNK):
                    nc.sync.dma_start(
                        ph[:, c], d_xbT_h[P * c:P * (c + 1),
                                          PANEL * p:PANEL * (p + 1)])
                    nc.sync.dma_start(
                        pls[:, c], d_xbT_ls[P * c:P * (c + 1),
                                            PANEL * p:PANEL * (p + 1)])
                nc.vector.tensor_scalar_mul(
                    phs.rearrange("p c s -> p (c s)"),
                    ph.rearrange("p c s -> p (c s)"), 1.0 / SPLIT)
                # k^T panel
                kps = miscP.tile([P, PANEL], f32, tag="misc")
                for c in range(NCHUNK):
                    nc.tensor.matmul(kps[:DH, :], wk_sb[:, c], ph[:, c],
                                     start=(c == 0), stop=(c == NCHUNK - 1))
                nc.scalar.copy(kT_sb[:, PANEL * p:PANEL * (p + 1)], kps[:DH, :])
                for l in range(p + 1, NLEV + 1):
                    emit_sim_tile(l, p, ph, pls, phs)
                emit_level_b(p + 1)

    nc.compile()
    return nc


def _gelu_exact_np(v):
    er = np.array([math.erf(float(t) / math.sqrt(2.0)) for t in v.ravel()],
                  dtype=np.float64).reshape(v.shape)
    return v * 0.5 * (1.0 + er)


def kernel(x, W_q, W_k, gain, bias, log_mix, log_scale):
    x = np.ascontiguousarray(np.asarray(x, dtype=np.float32))
    W_q = np.asarray(W_q, dtype=np.float32)
    W_k = np.asarray(W_k, dtype=np.float32)
    gain = np.asarray(gain, dtype=np.float32)
    bias = np.asarray(bias, dtype=np.float32)
    mix = float(1.0 / (1.0 + math.exp(-float(log_mix))))
    scale = float(np.log1p(np.exp(np.float32(log_scale))) + np.float32(0.01))

    if "prog" not in _prog_cache:
        _prog_cache["prog"] = _build_program()
    nc = _prog_cache["prog"]

    # ---- host-side layout prep ----
    xh = x.astype(np.float16)
    xls = ((x - xh.astype(np.float32)) * SPLIT).astype(np.float16)

    ident16 = np.eye(P, dtype=np.float16)
    identbf = np.eye(P, dtype=np.float32).astype(ml_dtypes.bfloat16)
    wqT = np.ascontiguousarray((W_q / math.sqrt(DH)).T.astype(np.float16))
    wkT = np.ascontiguousarray(W_k.T.astype(np.float16))
    grow = ((1.0 - mix) * gain).reshape(1, D).astype(np.float32)
    scalecol = np.full((P, 1), scale, dtype=np.float32)

    per_batch = {}
    for b in range(B):
        xbT_h = np.ascontiguousarray(xh[b].T)
        per_batch[b] = {
            "xbT_h": xbT_h,
            "xbT_ls": np.ascontiguousarray(xls[b].T),
            "xb_h": xh[b],
        }

    in_maps = []
    for core in range(NCORES):
        b, j = core // 4, core % 4
        rows = np.concatenate(
            [np.arange(P * (4 * l + j), P * (4 * l + j) + P) for l in range(NLEV)])
        xr = x[b][rows].reshape(NLEV, P, D)          # [lev, t, d] fp32
        xrh = xh[b][rows].reshape(NLEV, P, D)
        xrls = xls[b][rows].reshape(NLEV, P, D)
        xrT_h = np.ascontiguousarray(
            xrh.reshape(NLEV, P, NCHUNK, P).transpose(0, 2, 3, 1))
        xrT_ls = np.ascontiguousarray(
            xrls.reshape(NLEV, P, NCHUNK, P).transpose(0, 2, 3, 1))
        xrT_hs = (xrT_h.astype(np.float32) / SPLIT).astype(np.float16)
        xrm = (mix * xr * gain + bias).astype(np.float16)
        md = np.zeros((P, PANEL), dtype=np.float32)
        k_idx = np.arange(P)[:, None]
        s_idx = np.arange(PANEL)[None, :]
        md[s_idx >= k_idx + P * j] = MASKVAL
        in_maps.append({
            **per_batch[b],
            "xrT_h": xrT_h, "xrT_ls": xrT_ls, "xrT_hs": xrT_hs, "xrm": xrm,
            "maskdiag": md.astype(ml_dtypes.bfloat16),
            "wqT": wqT, "wkT": wkT,
            "ident16": ident16, "identbf": identbf,
            "grow": grow,
            "zerocol": np.zeros((P, 1), dtype=np.float32),
            "scalecol": scalecol,
            "eshift": np.full((P, 1), ESHIFT, dtype=np.float32),
            "onesrow": np.ones((1, P), dtype=np.float32),
        })

    res = run_bass_kernel_spmd(nc, in_maps, core_ids=list(range(NCORES)))
    _prog_cache["last_results"] = res

    out = np.empty((B, T, D), dtype=np.float32)
    for core in range(NCORES):
        b, j = core // 4, core % 4
        o = res.results[core]["out"]                 # [lev, t, d]
        for l in range(NLEV):
            r0 = P * (4 * l + j)
            out[b, r0:r0 + P, :] = o[l]

    # degenerate t=0 rows: uniform attention over ALL positions
    for b in range(B):
        msg0 = x[b].sum(axis=0, dtype=np.float32) * np.float32(1.0 / T)
        blended = np.float32(mix) * x[b, 0] + np.float32(1.0 - mix) * msg0
        pre = blended * gain + bias
        out[b, 0, :] = (_gelu_exact_np(pre.astype(np.float64))
                        * scale).astype(np.float32)
    return out



# revision 6
# speedup vs baseline: 1.2561x; 1.2561x over previous
"""Trainium2 Bass kernel for causal top-8 sparse attention (nn_DGN7).

Math (see reference):
  A    = top-8 strictly-causal neighbours of each row by x.x^T similarity
  attn = softmax over the selected scores, score = (x Wq^T)(x Wk^T)^T/sqrt(32)
  out  = gelu_exact((mix*x + (1-mix)*attn@x) * gain + bias) * (softplus+0.01)

Sharding: 8 cores; core i handles batch i//4 and, for every prefix level
l=1..8, the 128-row tile g = 4*(l-1) + (i%4).  Every core runs an identical
static program over strips of width 512*l (l=1..8); total causal area is
exactly balanced across cores.

Numerics:
  - similarity strip kept in units of 2048*x.x' (selection is scale
    invariant).  Main term (32h).(64h)' in fp16 (exact power-of-2 scalings
    of h=fp16(x)); hi/lo cross terms h.l' + l.h' (l = fp16((x-h)*2048)) via
    ONE fp8e4m3 DoubleRow matmul per 128-chunk (2x PE rate), i.e. sim costs
    16 fp16-equivalent chunk passes instead of 24.  Sim abs error ~1.3 strip
    units (~2e-5 in x.x'/32 units).
  - top-8 via DVE Max8 + match_replace (exact, first-match tie-break).
    Rows whose 8th/9th-candidate gap < 20 strip units are flagged (v8,v9
    exported) and recomputed exactly on the host (~70 of 8192 rows).
  - q/k/score/msg matmuls in fp16; softmax without max-shift (exp biased by
    -4); Z via ACT accum_out; normalisation after the msg matmul.
  - selection mask applied additively to the scores (0 / -3e38).
  - gain/bias/mix folded host-side: xrm = mix*x*gain + bias, gainb=(1-mix)*gain.
Host does layout prep (transposes/fp16/fp8 piece casts), the degenerate t=0
rows, and the flagged near-tie rows.
"""
import math
import numpy as np
import ml_dtypes

import concourse.bass as bass
import concourse.mybir as mybir
from concourse import bacc
from concourse.tile import TileContext
from concourse.bass_utils import run_bass_kernel_spmd

B, T, D = 2, 4096, 1024
DH = 32
P = 128
PANEL = 512
NLEV = 8
NPAN = 8
NCHUNK = D // P          # 8
NCORES = 8
FMIN = float(np.finfo(np.float32).min)
MASKTHR = -1e38
MASKVAL = -3.0e38        # finite in bf16 (FMIN would round to -inf)
SPLIT = 2048.0           # 2^11 lo-piece scale
ESHIFT = -4.0            # exp input bias (fp16 range safety)
GAPTHR = 20.0            # flag threshold, strip units (2048 * x.x')

f32 = mybir.dt.float32
f16 = mybir.dt.float16
bf16 = mybir.dt.bfloat16
f8 = mybir.dt.float8e4
DR = mybir.MatmulPerfMode.DoubleRow
FP8 = ml_dtypes.float8_e4m3

_prog_cache = {}


def _build_program(act_fn=None, use_dr=True):
    if act_fn is None:
        act_fn = mybir.ActivationFunctionType.Gelu
    nc = bacc.Bacc(trn_type="TRN2")

    # ---------------- DRAM I/O ----------------
    d_pan16 = nc.dram_tensor("pan16", [NPAN, P, NCHUNK, PANEL], f16,
                             kind="ExternalInput")      # (64h)^T panels
    d_pan8 = nc.dram_tensor("pan8", [NPAN, P, NCHUNK, 2, PANEL], f8,
                            kind="ExternalInput")       # (l8,h8)^T slabs
    d_xr16 = nc.dram_tensor("xr16", [NLEV, P, NCHUNK, P], f16,
                            kind="ExternalInput")       # (32h)^T own rows
    d_xr8 = nc.dram_tensor("xr8", [NLEV, P, NCHUNK, 2, P], f8,
                           kind="ExternalInput")        # (h8,l8)^T own rows
    d_xbh = nc.dram_tensor("xbh", [NPAN, P, 4, D], f16, kind="ExternalInput")
    d_xrm = nc.dram_tensor("xrm", [NLEV, P, D], f16, kind="ExternalInput")
    d_wq = nc.dram_tensor("wq", [P, NCHUNK, DH], f16, kind="ExternalInput")
    d_wk = nc.dram_tensor("wk", [P, NCHUNK, DH], f16, kind="ExternalInput")
    d_maskdiag = nc.dram_tensor("maskdiag", [P, PANEL], bf16, kind="ExternalInput")
    d_ident16 = nc.dram_tensor("ident16", [P, P], f16, kind="ExternalInput")
    d_identbf = nc.dram_tensor("identbf", [P, P], bf16, kind="ExternalInput")
    d_grow = nc.dram_tensor("grow", [1, D], f32, kind="ExternalInput")
    d_zerocol = nc.dram_tensor("zerocol", [P, 1], f32, kind="ExternalInput")
    d_scalecol = nc.dram_tensor("scalecol", [P, 1], f32, kind="ExternalInput")
    d_eshift = nc.dram_tensor("eshift", [P, 1], f32, kind="ExternalInput")
    d_onesrow = nc.dram_tensor("onesrow", [1, P], f32, kind="ExternalInput")
    d_out = nc.dram_tensor("out", [NLEV, P, D], f32, kind="ExternalOutput")
    d_v89 = nc.dram_tensor("v89", [NLEV, P, 2], f32, kind="ExternalOutput")

    with TileContext(nc) as tc:
        with tc.tile_pool(name="const", bufs=1) as cpool, \
             tc.tile_pool(name="strips", bufs=1) as spool, \
             tc.tile_pool(name="big", bufs=1) as bpool, \
             tc.tile_pool(name="panels", bufs=2) as ppool, \
             tc.tile_pool(name="attn", bufs=1) as apool, \
             tc.tile_pool(name="msgx", bufs=2) as mpool, \
             tc.tile_pool(name="work", bufs=2) as wpool, \
             tc.tile_pool(name="work1", bufs=1) as w1pool, \
             tc.tile_pool(name="simP", bufs=2, space="PSUM") as simP, \
             tc.tile_pool(name="miscP", bufs=2, space="PSUM") as miscP, \
             tc.tile_pool(name="tranP", bufs=2, space="PSUM") as tranP, \
             tc.tile_pool(name="msgP", bufs=2, space="PSUM") as msgP:

            # ---------------- panel 0 + own-row DMAs first ----------------
            def load_panel(p):
                t16 = ppool.tile([P, NCHUNK, PANEL], f16, tag="pan16")
                t8 = ppool.tile([P, NCHUNK, 2, PANEL], f8, tag="pan8")
                nc.sync.dma_start(t16, d_pan16[p])
                nc.scalar.dma_start(t8, d_pan8[p])
                return t16, t8

            cur = load_panel(0)

            xr16_sb, xr8_sb = [], []
            for l in range(NLEV):
                t16 = bpool.tile([P, NCHUNK, P], f16, tag=f"xr16_{l}",
                                 name=f"xr16_{l}")
                t8 = bpool.tile([P, NCHUNK, 2, P], f8, tag=f"xr8_{l}",
                                name=f"xr8_{l}")
                nc.gpsimd.dma_start(t16, d_xr16[l])
                nc.gpsimd.dma_start(t8, d_xr8[l])
                xr16_sb.append(t16)
                xr8_sb.append(t8)

            # ---------------- constants (scalar queue) ----------------
            wq_sb = cpool.tile([P, NCHUNK, DH], f16)
            wk_sb = cpool.tile([P, NCHUNK, DH], f16)
            nc.scalar.dma_start(wq_sb, d_wq.ap())
            nc.scalar.dma_start(wk_sb, d_wk.ap())
            ident16 = cpool.tile([P, P], f16)
            nc.scalar.dma_start(ident16, d_ident16.ap())
            identbf = cpool.tile([P, P], bf16)
            nc.scalar.dma_start(identbf, d_identbf.ap())
            maskdiag = cpool.tile([P, PANEL], bf16)
            nc.scalar.dma_start(maskdiag, d_maskdiag.ap())
            zerocol = cpool.tile([P, 1], f32)
            nc.scalar.dma_start(zerocol, d_zerocol.ap())
            scalecol = cpool.tile([P, 1], f32)
            nc.scalar.dma_start(scalecol, d_scalecol.ap())
            eshiftcol = cpool.tile([P, 1], f32)
            nc.scalar.dma_start(eshiftcol, d_eshift.ap())
            onesrow = cpool.tile([1, P], f32)
            nc.scalar.dma_start(onesrow, d_onesrow.ap())
            growrow = w1pool.tile([1, D], f32, tag="grow", name="growrow")
            nc.scalar.dma_start(growrow, d_grow.ap())

            kT_sb = cpool.tile([DH, T], f16)        # k^T, filled per panel
            gainb = cpool.tile([P, D], f16)
            strip = [spool.tile([P, PANEL * (l + 1)], f32, tag=f"strip{l}",
                                name=f"strip{l}")
                     for l in range(NLEV)]
            qT = [cpool.tile([DH, P], f16, tag=f"qT{l}", name=f"qT{l}")
                  for l in range(NLEV)]
            attnT = apool.tile([P, 4 * NLEV, P], f16, tag="attnT")

            def emit_sim_tile(l, p, p16, p8):
                ps = simP.tile([P, PANEL], f32, tag="sim")
                li = l - 1
                last = (p == l - 1)
                n = NCHUNK + NCHUNK + (1 if last else 0)
                i = 0
                for c in range(NCHUNK):
                    nc.tensor.matmul(ps, xr16_sb[li][:, c], p16[:, c],
                                     start=(i == 0), stop=(i == n - 1))
                    i += 1
                if use_dr:
                    for c in range(NCHUNK):
                        nc.tensor.matmul(ps, xr8_sb[li][:, c], p8[:, c],
                                         start=False, stop=(i == n - 1),
                                         perf_mode=DR)
                        i += 1
                else:
                    for c in range(NCHUNK):
                        for s2 in range(2):
                            nc.tensor.matmul(ps, xr8_sb[li][:, c, s2],
                                             p8[:, c, s2],
                                             start=False,
                                             stop=(i == n - 1 and s2 == 1))
                    i += 1
                if last:
                    nc.tensor.matmul(ps, identbf, maskdiag,
                                     start=False, stop=True)
                nc.scalar.copy(strip[li][:, PANEL * p:PANEL * (p + 1)], ps)

            def issue_selection(l):
                """DVE: top-8 select, flag columns, additive mask (in place)"""
                li = l - 1
                st = strip[li]
                top8 = w1pool.tile([P, 8], f32, tag="top8")
                nc.vector.max(out=top8, in_=st)
                nc.vector.match_replace(out=st, in_to_replace=top8,
                                        in_values=st, imm_value=FMIN)
                v9c = w1pool.tile([P, 1], f32, tag="v9")
                nc.vector.tensor_reduce(out=v9c, in_=st,
                                        op=mybir.AluOpType.max,
                                        axis=mybir.AxisListType.X)
                v8c = w1pool.tile([P, 1], f32, tag="v8")
                nc.vector.tensor_reduce(out=v8c, in_=top8,
                                        op=mybir.AluOpType.min,
                                        axis=mybir.AxisListType.X)
                nc.gpsimd.dma_start(d_v89[li][:, 0:1], v8c)
                nc.gpsimd.dma_start(d_v89[li][:, 1:2], v9c)
                nc.vector.tensor_scalar(st, st, MASKTHR, scalar2=MASKVAL,
                                        op0=mybir.AluOpType.is_gt,
                                        op1=mybir.AluOpType.mult)

            def level_compute(l):
                """scores/exp/attn^T + msg + out for level l (mask ready)"""
                li = l - 1
                st = strip[li]
                # xrm for the out stage (issued early, gpsimd queue)
                xrm = w1pool.tile([P, D], f16, tag="xrm")
                nc.gpsimd.dma_start(xrm, d_xrm[li])
                # --- q^T for this level ---
                qps = miscP.tile([P, PANEL], f32, tag="misc")
                for c in range(NCHUNK):
                    nc.tensor.matmul(qps[:DH, :P], wq_sb[:, c],
                                     xr16_sb[li][:, c],
                                     start=(c == 0), stop=(c == NCHUNK - 1))
                nc.scalar.copy(qT[li], qps[:DH, :P])
                zcols = w1pool.tile([P, NLEV], f32, tag="zcols")
                mp0 = msgP.tile([P, PANEL], f32, tag="msg")
                mp1 = msgP.tile([P, PANEL], f32, tag="msg")
                mps = [mp0, mp1]
                nblk = 4 * l
                dmaq = [nc.scalar, nc.gpsimd, nc.sync]

                def load_xbh(c):
                    xbh = mpool.tile([P, 4, D], f16, tag="xbh")
                    dmaq[c % 3].dma_start(xbh, d_xbh[c])
                    return xbh

                xbh_next = load_xbh(0)
                for c in range(l):
                    xbh = xbh_next
                    if c + 1 < l:
                        xbh_next = load_xbh(c + 1)
                    sps = miscP.tile([P, PANEL], f32, tag="misc")
                    nc.tensor.matmul(sps, qT[li],
                                     kT_sb[:, PANEL * c:PANEL * (c + 1)],
                                     start=True, stop=(c != l - 1))
                    if c == l - 1:
                        nc.tensor.matmul(sps, identbf, maskdiag,
                                         start=False, stop=True)
                    nc.vector.tensor_add(sps, sps,
                                         st[:, PANEL * c:PANEL * (c + 1)])
                    au = wpool.tile([P, PANEL], f16, tag="au")
                    nc.scalar.activation(au, sps,
                                         mybir.ActivationFunctionType.Exp,
                                         bias=eshiftcol, scale=1.0,
                                         accum_out=zcols[:, c:c + 1])
                    tp = tranP.tile([P, PANEL], f16, tag="tran")
                    for q in range(4):
                        nc.tensor.matmul(tp[:, P * q:P * (q + 1)],
                                         au[:, P * q:P * (q + 1)], ident16,
                                         is_transpose=True,
                                         start=(q == 0), stop=(q == 3))
                    nc.scalar.copy(
                        attnT[:, 4 * c:4 * (c + 1)].rearrange("p b t -> p (b t)"),
                        tp)
                    for sb in range(4):
                        blk = 4 * c + sb
                        for k in range(2):
                            nc.tensor.matmul(
                                mps[k], attnT[:, blk],
                                xbh[:, sb, PANEL * k:PANEL * (k + 1)],
                                start=(blk == 0), stop=(blk == nblk - 1))
                # --- Z -> 1/Z (per-partition column) ---
                zsum = w1pool.tile([P, 1], f32, tag="zsum")
                nc.vector.tensor_reduce(
                    out=zsum, in_=zcols[:, :l], op=mybir.AluOpType.add,
                    axis=mybir.AxisListType.X)
                nc.vector.tensor_scalar_max(zsum, zsum, 1e-30)
                zrec = w1pool.tile([P, 1], f32, tag="zrec")
                nc.vector.reciprocal(zrec, zsum)
                # --- out stage (t-major) ---
                for k in range(2):
                    sl = slice(PANEL * k, PANEL * (k + 1))
                    gh = w1pool.tile([P, PANEL], f32, tag=f"g{k}",
                                     name=f"g{k}")
                    nc.vector.tensor_scalar_mul(gh, mps[k], zrec)
                    nc.vector.tensor_mul(gh, gh, gainb[:, sl])
                    nc.vector.tensor_add(gh, gh, xrm[:, sl])
                    nc.scalar.activation(gh, gh, act_fn, bias=zerocol, scale=1.0)
                    nc.vector.tensor_scalar_mul(gh, gh, scalecol)
                    nc.sync.dma_start(d_out[li][:, sl], gh)

            # ---------------- main pipeline ----------------
            for p in range(NPAN):
                nxt = load_panel(p + 1) if p + 1 < NPAN else None
                if p >= 1:
                    issue_selection(p)
                p16, p8 = cur
                # level p+1's last tile first, so its strip completes early
                emit_sim_tile(p + 1, p, p16, p8)
                # k^T panel
                kps = miscP.tile([P, PANEL], f32, tag="misc")
                for c in range(NCHUNK):
                    nc.tensor.matmul(kps[:DH, :], wk_sb[:, c], p16[:, c],
                                     start=(c == 0), stop=(c == NCHUNK - 1))
                nc.scalar.copy(kT_sb[:, PANEL * p:PANEL * (p + 1)], kps[:DH, :])
                for l in range(p + 2, NLEV + 1):
                    emit_sim_tile(l, p, p16, p8)
                if p == 0:
                    # (1-mix)*gain broadcast to all partitions via K=1 matmuls
                    for k in range(2):
                        gps = msgP.tile([P, PANEL], f32, tag="msg")
                        nc.tensor.matmul(gps, onesrow,
                                         growrow[:, PANEL * k:PANEL * (k + 1)],
                                         start=True, stop=True)
                        nc.scalar.copy(gainb[:, PANEL * k:PANEL * (k + 1)], gps)
                if p >= 1:
                    level_compute(p)
                cur = nxt
            issue_selection(NLEV)
            level_compute(NLEV)

    nc.compile()
    return nc


def _gelu_exact_np(v):
    er = np.array([math.erf(float(t) / math.sqrt(2.0)) for t in v.ravel()],
                  dtype=np.float64).reshape(v.shape)
    return v * 0.5 * (1.0 + er)


def _fix_row(out, xb, W_q, W_k, gain, bias, mix, scale, t):
    """Recompute row t of batch xb exactly (host, fp32 selection/fp64 tail)."""
    kk = min(8, t)
    if kk == 0:
        return  # t=0 handled by caller
    srow = xb[:t] @ xb[t]                       # fp32 similarities (j < t)
    idx = np.argsort(-srow, kind="stable")[:kk]
    q = (xb[t:t + 1] @ W_q.T).astype(np.float64)[0] / math.sqrt(DH)
    kv = (xb[idx] @ W_k.T).astype(np.float64)
    sc = kv @ q
    sc -= sc.max()
    e = np.exp(sc)
    a = e / e.sum()
    msg = a @ xb[idx].astype(np.float64)
    blended = mix * xb[t].astype(np.float64) + (1.0 - mix) * msg
    pre = blended * gain.astype(np.float64) + bias.astype(np.float64)
    out[t] = (_gelu_exact_np(pre) * scale).astype(np.float32)


def kernel(x, W_q, W_k, gain, bias, log_mix, log_scale):
    x = np.ascontiguousarray(np.asarray(x, dtype=np.float32))
    W_q = np.asarray(W_q, dtype=np.float32)
    W_k = np.asarray(W_k, dtype=np.float32)
    gain = np.asarray(gain, dtype=np.float32)
    bias = np.asarray(bias, dtype=np.float32)
    mix = float(1.0 / (1.0 + math.exp(-float(log_mix))))
    scale = float(np.log1p(np.exp(np.float32(log_scale))) + np.float32(0.01))

    if "prog" not in _prog_cache:
        _prog_cache["prog"] = _build_program()
    nc = _prog_cache["prog"]

    # ---- host-side layout prep ----
    xh = x.astype(np.float16)
    hf = xh.astype(np.float32)
    xl = ((x - hf) * SPLIT).astype(np.float16)
    h32 = (hf * 32.0).astype(np.float16)     # exact power-of-2 scalings
    h64 = (hf * 64.0).astype(np.float16)
    h8 = xh.astype(FP8)
    l8 = xl.astype(FP8)

    ident16 = np.eye(P, dtype=np.float16)
    identbf = np.eye(P, dtype=np.float32).astype(ml_dtypes.bfloat16)
    wq = np.ascontiguousarray(
        (W_q / (32.0 * math.sqrt(DH))).T.astype(np.float16)
        .reshape(NCHUNK, P, DH).transpose(1, 0, 2))
    wk = np.ascontiguousarray(
        (W_k / 64.0).T.astype(np.float16)
        .reshape(NCHUNK, P, DH).transpose(1, 0, 2))
    grow = ((1.0 - mix) * gain).reshape(1, D).astype(np.float32)
    scalecol = np.full((P, 1), scale, dtype=np.float32)

    per_batch = {}
    for b in range(B):
        pan16 = np.ascontiguousarray(
            h64[b].T.reshape(NCHUNK, P, NPAN, PANEL).transpose(2, 1, 0, 3))
        l8T = l8[b].T.reshape(NCHUNK, P, NPAN, PANEL)
        h8T = h8[b].T.reshape(NCHUNK, P, NPAN, PANEL)
        pan8 = np.ascontiguousarray(
            np.stack([l8T, h8T], axis=2).transpose(3, 1, 0, 2, 4))
        xbh = np.ascontiguousarray(
            xh[b].reshape(NPAN, 4, P, D).transpose(0, 2, 1, 3))
        per_batch[b] = {"pan16": pan16, "pan8": pan8, "xbh": xbh}

    in_maps = []
    for core in range(NCORES):
        b, j = core // 4, core % 4
        rows = np.concatenate(
            [np.arange(P * (4 * l + j), P * (4 * l + j) + P) for l in range(NLEV)])
        xr = x[b][rows].reshape(NLEV, P, D)          # [lev, t, d] fp32
        xr16 = np.ascontiguousarray(
            h32[b][rows].reshape(NLEV, P, NCHUNK, P).transpose(0, 3, 2, 1))
        h8r = h8[b][rows].reshape(NLEV, P, NCHUNK, P)
        l8r = l8[b][rows].reshape(NLEV, P, NCHUNK, P)
        xr8 = np.ascontiguousarray(
            np.stack([h8r, l8r], axis=3).transpose(0, 4, 2, 3, 1))
        xrm = (mix * xr * gain + bias).astype(np.float16)
        md = np.zeros((P, PANEL), dtype=np.float32)
        k_idx = np.arange(P)[:, None]
        s_idx = np.arange(PANEL)[None, :]
        md[s_idx >= k_idx + P * j] = MASKVAL
        in_maps.append({
            **per_batch[b],
            "xr16": xr16, "xr8": xr8, "xrm": xrm,
            "maskdiag": md.astype(ml_dtypes.bfloat16),
            "wq": wq, "wk": wk,
            "ident16": ident16, "identbf": identbf,
            "grow": grow,
            "zerocol": np.zeros((P, 1), dtype=np.float32),
            "scalecol": scalecol,
            "eshift": np.full((P, 1), ESHIFT, dtype=np.float32),
            "onesrow": np.ones((1, P), dtype=np.float32),
        })

    res = run_bass_kernel_spmd(nc, in_maps, core_ids=list(range(NCORES)))
    _prog_cache["last_results"] = res

    out = np.empty((B, T, D), dtype=np.float32)
    flagged = []
    for core in range(NCORES):
        b, j = core // 4, core % 4
        o = res.results[core]["out"]                 # [lev, t, d]
        v89 = res.results[core]["v89"]               # [lev, t, 2]
        for l in range(NLEV):
            r0 = P * (4 * l + j)
            out[b, r0:r0 + P, :] = o[l]
            gap = v89[l, :, 0].astype(np.float64) - v89[l, :, 1].astype(np.float64)
            for r in np.nonzero(gap < GAPTHR)[0]:
                flagged.append((b, r0 + int(r)))

    # near-tie rows: recompute exactly on host (selection ambiguous on device)
    for b, t in flagged:
        _fix_row(out[b], x[b], W_q, W_k, gain, bias, mix, scale, t)

    # degenerate t=0 rows: uniform attention over ALL positions
    for b in range(B):
        msg0 = x[b].sum(axis=0, dtype=np.float32) * np.float32(1.0 / T)
        blended = np.float32(mix) * x[b, 0] + np.float32(1.0 - mix) * msg0
        pre = blended * gain + bias
        out[b, 0, :] = (_gelu_exact_np(pre.astype(np.float64))
                        * scale).astype(np.float32)
    return out


# revision 17
# speedup vs baseline: 1.2937x; 1.0300x over previous
"""Trainium2 Bass kernel for causal top-8 sparse attention (nn_DGN7).

Math (see reference):
  A    = top-8 strictly-causal neighbours of each row by x.x^T similarity
  attn = softmax over the selected scores, score = (x Wq^T)(x Wk^T)^T/sqrt(32)
  out  = gelu_exact((mix*x + (1-mix)*attn@x) * gain + bias) * (softplus+0.01)

Sharding: 8 cores; core i handles batch i//4 and, for every prefix level
l=1..8, the 128-row tile g = 4*(l-1) + (i%4).  Every core runs an identical
static program over strips of width 512*l (l=1..8); total causal area is
exactly balanced across cores.

Numerics:
  - similarity strip kept in units of 2048*x.x' (selection is scale
    invariant).  Main term (32h).(64h)' in fp16 (exact power-of-2 scalings
    of h=fp16(x)); hi/lo cross terms h.l' + l.h' (l = fp16((x-h)*2048)) via
    ONE fp8e4m3 DoubleRow matmul per 128-chunk (2x PE rate), i.e. sim costs
    16 fp16-equivalent chunk passes instead of 24.  Sim abs error ~1.3 strip
    units (~2e-5 in x.x'/32 units).
  - top-8 via DVE Max8 + match_replace (exact, first-match tie-break).
    Rows whose 8th/9th-candidate gap < 20 strip units are flagged (v8,v9
    exported) and recomputed exactly on the host (~70 of 8192 rows).
  - q/k/score/msg matmuls in fp16; softmax without max-shift (exp biased by
    -4); Z via ACT accum_out; normalisation after the msg matmul.
  - selection mask applied additively to the scores (0 / -3e38).
  - gain/bias/mix folded host-side: xrm = mix*x*gain + bias, gainb=(1-mix)*gain.
Host does layout prep (transposes/fp16/fp8 piece casts), the degenerate t=0
rows, and the flagged near-tie rows.
"""
import math
import numpy as np
import ml_dtypes

import concourse.bass as bass
import concourse.mybir as mybir
from concourse import bacc
from concourse.tile import TileContext
from concourse.bass_utils import run_bass_kernel_spmd

B, T, D = 2, 4096, 1024
DH = 32
P = 128
PANEL = 512
NLEV = 8
NPAN = 8
NCHUNK = D // P          # 8
NCORES = 8
FMIN = float(np.finfo(np.float32).min)
MASKTHR = -1e38
MASKVAL = -3.0e38        # finite in bf16 (FMIN would round to -inf)
SPLIT = 2048.0           # 2^11 lo-piece scale
ESHIFT = -4.0            # exp input bias (fp16 range safety)
GAPTHR = 20.0            # flag threshold, strip units (2048 * x.x')

f32 = mybir.dt.float32
f16 = mybir.dt.float16
bf16 = mybir.dt.bfloat16
f8 = mybir.dt.float8e4
DR = mybir.MatmulPerfMode.DoubleRow
FP8 = ml_dtypes.float8_e4m3

_prog_cache = {}


def _build_program(act_fn=None, use_dr=True):
    if act_fn is None:
        act_fn = mybir.ActivationFunctionType.Gelu
    nc = bacc.Bacc(trn_type="TRN2")

    # ---------------- DRAM I/O ----------------
    d_pan16 = nc.dram_tensor("pan16", [NPAN, P, NCHUNK, PANEL], f16,
                             kind="ExternalInput")      # (64h)^T panels
    d_pan8 = nc.dram_tensor("pan8", [NPAN, P, NCHUNK, 2, PANEL], f8,
                            kind="ExternalInput")       # (l8,h8)^T slabs
    d_xr16 = nc.dram_tensor("xr16", [NLEV, P, NCHUNK, P], f16,
                            kind="ExternalInput")       # (32h)^T own rows
    d_xr8 = nc.dram_tensor("xr8", [NLEV, P, NCHUNK, 2, P], f8,
                           kind="ExternalInput")        # (h8,l8)^T own rows
    d_xbh = nc.dram_tensor("xbh", [NPAN, P, 4, D], f16, kind="ExternalInput")
    d_xrm = nc.dram_tensor("xrm", [NLEV, P, D], f16, kind="ExternalInput")
    d_wq = nc.dram_tensor("wq", [P, NCHUNK, DH], f16, kind="ExternalInput")
    d_wk = nc.dram_tensor("wk", [P, NCHUNK, DH], f16, kind="ExternalInput")
    d_maskdiag = nc.dram_tensor("maskdiag", [P, PANEL], bf16, kind="ExternalInput")
    d_ident16 = nc.dram_tensor("ident16", [P, P], f16, kind="ExternalInput")
    d_identbf = nc.dram_tensor("identbf", [P, P], bf16, kind="ExternalInput")
    d_zerocol = nc.dram_tensor("zerocol", [P, 1], f32, kind="ExternalInput")
    d_scalecol = nc.dram_tensor("scalecol", [P, 1], f32, kind="ExternalInput")
    d_eshift = nc.dram_tensor("eshift", [P, 1], f32, kind="ExternalInput")
    d_out = nc.dram_tensor("out", [NLEV, P, D], f32, kind="ExternalOutput")
    d_v89 = nc.dram_tensor("v89", [NLEV, P, 2], f32, kind="ExternalOutput")

    with TileContext(nc) as tc:
        with tc.tile_pool(name="const", bufs=1) as cpool, \
             tc.tile_pool(name="strips", bufs=1) as spool, \
             tc.tile_pool(name="big", bufs=1) as bpool, \
             tc.tile_pool(name="panels", bufs=2) as ppool, \
             tc.tile_pool(name="attn", bufs=1) as apool, \
             tc.tile_pool(name="msgx", bufs=2) as mpool, \
             tc.tile_pool(name="work", bufs=2) as wpool, \
             tc.tile_pool(name="work1", bufs=1) as w1pool, \
             tc.tile_pool(name="simP", bufs=2, space="PSUM") as simP, \
             tc.tile_pool(name="miscP", bufs=2, space="PSUM") as miscP, \
             tc.tile_pool(name="tranP", bufs=2, space="PSUM") as tranP, \
             tc.tile_pool(name="msgP", bufs=2, space="PSUM") as msgP:

            # ---------------- panel 0 + own-row DMAs first ----------------
            def load_panel(p):
                t16 = ppool.tile([P, NCHUNK, PANEL], f16, tag="pan16")
                t8 = ppool.tile([P, NCHUNK, 2, PANEL], f8, tag="pan8")
                nc.sync.dma_start(t16, d_pan16[p])
                (nc.scalar if p == 0 else nc.sync).dma_start(t8, d_pan8[p])
                return t16, t8

            cur = load_panel(0)

            xr16_sb, xr8_sb = [], []
            for l in range(NLEV):
                t16 = bpool.tile([P, NCHUNK, P], f16, tag=f"xr16_{l}",
                                 name=f"xr16_{l}")
                t8 = bpool.tile([P, NCHUNK, 2, P], f8, tag=f"xr8_{l}",
                                name=f"xr8_{l}")
                nc.gpsimd.dma_start(t16, d_xr16[l])
                nc.gpsimd.dma_start(t8, d_xr8[l])
                xr16_sb.append(t16)
                xr8_sb.append(t8)

            # ---------------- constants (scalar queue) ----------------
            wq_sb = cpool.tile([P, NCHUNK, DH], f16)
            wk_sb = cpool.tile([P, NCHUNK, DH], f16)
            nc.scalar.dma_start(wq_sb, d_wq.ap())
            nc.scalar.dma_start(wk_sb, d_wk.ap())
            ident16 = cpool.tile([P, P], f16)
            nc.scalar.dma_start(ident16, d_ident16.ap())
            identbf = cpool.tile([P, P], bf16)
            nc.scalar.dma_start(identbf, d_identbf.ap())
            maskdiag = cpool.tile([P, PANEL], bf16)
            nc.scalar.dma_start(maskdiag, d_maskdiag.ap())
            zerocol = cpool.tile([P, 1], f32)
            nc.scalar.dma_start(zerocol, d_zerocol.ap())
            scalecol = cpool.tile([P, 1], f32)
            nc.scalar.dma_start(scalecol, d_scalecol.ap())
            eshiftcol = cpool.tile([P, 1], f32)
            nc.scalar.dma_start(eshiftcol, d_eshift.ap())

            kT_sb = cpool.tile([DH, T], f16)        # k^T, filled per panel
            strip = [spool.tile([P, PANEL * (l + 1)], f32, tag=f"strip{l}",
                                name=f"strip{l}")
                     for l in range(NLEV)]
            qT = [cpool.tile([DH, P], f16, tag=f"qT{l}", name=f"qT{l}")
                  for l in range(NLEV)]
            attnT = apool.tile([P, 4 * NLEV, P], f16, tag="attnT")

            def emit_sim_tile(l, p, p16, p8, critical=False):
                ps = simP.tile([P, PANEL], f32, tag="sim")
                li = l - 1
                last = (p == l - 1)
                n = NCHUNK + NCHUNK + (1 if last else 0)
                i = 0
                for c in range(NCHUNK):
                    nc.tensor.matmul(ps, xr16_sb[li][:, c], p16[:, c],
                                     start=(i == 0), stop=(i == n - 1))
                    i += 1
                if use_dr:
                    for c in range(NCHUNK):
                        nc.tensor.matmul(ps, xr8_sb[li][:, c], p8[:, c],
                                         start=False, stop=(i == n - 1),
                                         perf_mode=DR)
                        i += 1
                else:
                    for c in range(NCHUNK):
                        for s2 in range(2):
                            nc.tensor.matmul(ps, xr8_sb[li][:, c, s2],
                                             p8[:, c, s2],
                                             start=False,
                                             stop=(i == n - 1 and s2 == 1))
                    i += 1
                if last:
                    nc.tensor.matmul(ps, identbf, maskdiag,
                                     start=False, stop=True)
                if critical:
                    # last tile of this level's strip: copy on DVE so the
                    # selection chain isn't stuck behind the ACT queue
                    nc.vector.tensor_copy(
                        strip[li][:, PANEL * p:PANEL * (p + 1)], ps)
                else:
                    nc.scalar.copy(strip[li][:, PANEL * p:PANEL * (p + 1)], ps)

            def issue_selection(l):
                """DVE: top-8 select, flag columns, additive mask (in place)"""
                li = l - 1
                st = strip[li]
                top8 = w1pool.tile([P, 8], f32, tag="top8")
                nc.vector.max(out=top8, in_=st)
                nc.vector.match_replace(out=st, in_to_replace=top8,
                                        in_values=st, imm_value=FMIN)
                v9c = w1pool.tile([P, 1], f32, tag="v9")
                nc.vector.tensor_reduce(out=v9c, in_=st,
                                        op=mybir.AluOpType.max,
                                        axis=mybir.AxisListType.X)
                v8c = w1pool.tile([P, 1], f32, tag="v8")
                nc.vector.tensor_reduce(out=v8c, in_=top8,
                                        op=mybir.AluOpType.min,
                                        axis=mybir.AxisListType.X)
                nc.gpsimd.dma_start(d_v89[li][:, 0:1], v8c)
                nc.gpsimd.dma_start(d_v89[li][:, 1:2], v9c)
                nc.vector.tensor_scalar(st, st, MASKTHR, scalar2=MASKVAL,
                                        op0=mybir.AluOpType.is_gt,
                                        op1=mybir.AluOpType.mult)

            def load_xbh(c):
                xbh = mpool.tile([P, 4, D], f16, tag="xbh")
                (nc.gpsimd if c % 2 else nc.sync).dma_start(xbh, d_xbh[c])
                return xbh

            def level_compute(l, xbh0):
                """scores/exp/attn^T + msg + out for level l (mask ready)"""
                li = l - 1
                st = strip[li]
                # xrm for the out stage (issued early, gpsimd queue)
                xrm = w1pool.tile([P, D], f16, tag="xrm")
                nc.gpsimd.dma_start(xrm, d_xrm[li])
                # --- q^T for this level ---
                qps = miscP.tile([P, PANEL], f32, tag="misc")
                for c in range(NCHUNK):
                    nc.tensor.matmul(qps[:DH, :P], wq_sb[:, c],
                                     xr16_sb[li][:, c],
                                     start=(c == 0), stop=(c == NCHUNK - 1))
                nc.scalar.copy(qT[li], qps[:DH, :P])
                zcols = w1pool.tile([P, NLEV], f32, tag="zcols")
                mp0 = msgP.tile([P, PANEL], f32, tag="msg")
                mp1 = msgP.tile([P, PANEL], f32, tag="msg")
                mps = [mp0, mp1]
                nblk = 4 * l

                xbh_next = xbh0
                for c in range(l):
                    xbh = xbh_next
                    if c + 1 < l:
                        xbh_next = load_xbh(c + 1)
                    sps = miscP.tile([P, PANEL], f32, tag="misc")
                    nc.tensor.matmul(sps, qT[li],
                                     kT_sb[:, PANEL * c:PANEL * (c + 1)],
                                     start=True, stop=(c != l - 1))
                    if c == l - 1:
                        nc.tensor.matmul(sps, identbf, maskdiag,
                                         start=False, stop=True)
                    nc.vector.tensor_add(sps, sps,
                                         st[:, PANEL * c:PANEL * (c + 1)])
                    au = wpool.tile([P, PANEL], f16, tag="au")
                    nc.scalar.activation(au, sps,
                                         mybir.ActivationFunctionType.Exp,
                                         bias=eshiftcol, scale=1.0,
                                         accum_out=zcols[:, c:c + 1])
                    tp = tranP.tile([P, PANEL], f16, tag="tran")
                    for q in range(4):
                        nc.tensor.matmul(tp[:, P * q:P * (q + 1)],
                                         au[:, P * q:P * (q + 1)], ident16,
                                         is_transpose=True,
                                         start=(q == 0), stop=(q == 3))
                    nc.scalar.copy(
                        attnT[:, 4 * c:4 * (c + 1)].rearrange("p b t -> p (b t)"),
                        tp)
                    for sb in range(4):
                        blk = 4 * c + sb
                        for k in range(2):
                            nc.tensor.matmul(
                                mps[k], attnT[:, blk],
                                xbh[:, sb, PANEL * k:PANEL * (k + 1)],
                                start=(blk == 0), stop=(blk == nblk - 1))
                # --- Z -> 1/Z (per-partition column) ---
                zsum = w1pool.tile([P, 1], f32, tag="zsum")
                nc.vector.tensor_reduce(
                    out=zsum, in_=zcols[:, :l], op=mybir.AluOpType.add,
                    axis=mybir.AxisListType.X)
                nc.vector.tensor_scalar_max(zsum, zsum, 1e-30)
                zrec = w1pool.tile([P, 1], f32, tag="zrec")
                nc.vector.reciprocal(zrec, zsum)
                # --- out stage (t-major); (1-mix)*gain pre-folded into xbh ---
                for k in range(2):
                    sl = slice(PANEL * k, PANEL * (k + 1))
                    gh = w1pool.tile([P, PANEL], f32, tag=f"g{k}",
                                     name=f"g{k}")
                    nc.vector.tensor_scalar_mul(gh, mps[k], zrec)
                    nc.vector.tensor_add(gh, gh, xrm[:, sl])
                    nc.scalar.activation(gh, gh, act_fn, bias=zerocol, scale=1.0)
                    nc.vector.tensor_scalar_mul(gh, gh, scalecol)
                    nc.sync.dma_start(d_out[li][:, sl], gh)

            # ---------------- main pipeline ----------------
            for p in range(NPAN):
                nxt = load_panel(p + 1) if p + 1 < NPAN else None
                xbh0 = load_xbh(0) if p >= 1 else None
                p16, p8 = cur
                # level p+1's last tile first, so its strip completes early
                # and its selection overlaps the rest of this iteration
                emit_sim_tile(p + 1, p, p16, p8, critical=True)
                issue_selection(p + 1)
                # k^T panel
                kps = miscP.tile([P, PANEL], f32, tag="misc")
                for c in range(NCHUNK):
                    nc.tensor.matmul(kps[:DH, :], wk_sb[:, c], p16[:, c],
                                     start=(c == 0), stop=(c == NCHUNK - 1))
                nc.scalar.copy(kT_sb[:, PANEL * p:PANEL * (p + 1)], kps[:DH, :])
                for l in range(p + 2, NLEV + 1):
                    emit_sim_tile(l, p, p16, p8)
                if p >= 1:
                    level_compute(p, xbh0)
                cur = nxt
            level_compute(NLEV, load_xbh(0))

    nc.compile()
    return nc


def _gelu_exact_np(v):
    er = np.array([math.erf(float(t) / math.sqrt(2.0)) for t in v.ravel()],
                  dtype=np.float64).reshape(v.shape)
    return v * 0.5 * (1.0 + er)


def _fix_row(out, xb, W_q, W_k, gain, bias, mix, scale, t):
    """Recompute row t of batch xb exactly (host, fp32 selection/fp64 tail)."""
    kk = min(8, t)
    if kk == 0:
        return  # t=0 handled by caller
    srow = xb[:t] @ xb[t]                       # fp32 similarities (j < t)
    idx = np.argsort(-srow, kind="stable")[:kk]
    q = (xb[t:t + 1] @ W_q.T).astype(np.float64)[0] / math.sqrt(DH)
    kv = (xb[idx] @ W_k.T).astype(np.float64)
    sc = kv @ q
    sc -= sc.max()
    e = np.exp(sc)
    a = e / e.sum()
    msg = a @ xb[idx].astype(np.float64)
    blended = mix * xb[t].astype(np.float64) + (1.0 - mix) * msg
    pre = blended * gain.astype(np.float64) + bias.astype(np.float64)
    out[t] = (_gelu_exact_np(pre) * scale).astype(np.float32)


def kernel(x, W_q, W_k, gain, bias, log_mix, log_scale):
    x = np.ascontiguousarray(np.asarray(x, dtype=np.float32))
    W_q = np.asarray(W_q, dtype=np.float32)
    W_k = np.asarray(W_k, dtype=np.float32)
    gain = np.asarray(gain, dtype=np.float32)
    bias = np.asarray(bias, dtype=np.float32)
    mix = float(1.0 / (1.0 + math.exp(-float(log_mix))))
    scale = float(np.log1p(np.exp(np.float32(log_scale))) + np.float32(0.01))

    if "prog" not in _prog_cache:
        _prog_cache["prog"] = _build_program()
    nc = _prog_cache["prog"]

    # ---- host-side layout prep ----
    xh = x.astype(np.float16)
    hf = xh.astype(np.float32)
    xl = ((x - hf) * SPLIT).astype(np.float16)
    h32 = (hf * 32.0).astype(np.float16)     # exact power-of-2 scalings
    h64 = (hf * 64.0).astype(np.float16)
    h8 = xh.astype(FP8)
    l8 = xl.astype(FP8)

    ident16 = np.eye(P, dtype=np.float16)
    identbf = np.eye(P, dtype=np.float32).astype(ml_dtypes.bfloat16)
    wq = np.ascontiguousarray(
        (W_q / (32.0 * math.sqrt(DH))).T.astype(np.float16)
        .reshape(NCHUNK, P, DH).transpose(1, 0, 2))
    wk = np.ascontiguousarray(
        (W_k / 64.0).T.astype(np.float16)
        .reshape(NCHUNK, P, DH).transpose(1, 0, 2))
    scalecol = np.full((P, 1), scale, dtype=np.float32)
    gainb = ((1.0 - mix) * gain).astype(np.float32)   # folded into xbh

    per_batch = {}
    for b in range(B):
        pan16 = np.ascontiguousarray(
            h64[b].T.reshape(NCHUNK, P, NPAN, PANEL).transpose(2, 1, 0, 3))
        l8T = l8[b].T.reshape(NCHUNK, P, NPAN, PANEL)
        h8T = h8[b].T.reshape(NCHUNK, P, NPAN, PANEL)
        pan8 = np.ascontiguousarray(
            np.stack([l8T, h8T], axis=2).transpose(3, 1, 0, 2, 4))
        xbh = np.ascontiguousarray(
            (x[b] * gainb).astype(np.float16)
            .reshape(NPAN, 4, P, D).transpose(0, 2, 1, 3))
        per_batch[b] = {"pan16": pan16, "pan8": pan8, "xbh": xbh}

    in_maps = []
    for core in range(NCORES):
        b, j = core // 4, core % 4
        rows = np.concatenate(
            [np.arange(P * (4 * l + j), P * (4 * l + j) + P) for l in range(NLEV)])
        xr = x[b][rows].reshape(NLEV, P, D)          # [lev, t, d] fp32
        xr16 = np.ascontiguousarray(
            h32[b][rows].reshape(NLEV, P, NCHUNK, P).transpose(0, 3, 2, 1))
        h8r = h8[b][rows].reshape(NLEV, P, NCHUNK, P)
        l8r = l8[b][rows].reshape(NLEV, P, NCHUNK, P)
        xr8 = np.ascontiguousarray(
            np.stack([h8r, l8r], axis=3).transpose(0, 4, 2, 3, 1))
        xrm = (mix * xr * gain + bias).astype(np.float16)
        md = np.zeros((P, PANEL), dtype=np.float32)
        k_idx = np.arange(P)[:, None]
        s_idx = np.arange(PANEL)[None, :]
        md[s_idx >= k_idx + P * j] = MASKVAL
        in_maps.append({
            **per_batch[b],
            "xr16": xr16, "xr8": xr8, "xrm": xrm,
            "maskdiag": md.astype(ml_dtypes.bfloat16),
            "wq": wq, "wk": wk,
            "ident16": ident16, "identbf": identbf,
            "zerocol": np.zeros((P, 1), dtype=np.float32),
            "scalecol": scalecol,
            "eshift": np.full((P, 1), ESHIFT, dtype=np.float32),
        })

    res = run_bass_kernel_spmd(nc, in_maps, core_ids=list(range(NCORES)))
    _prog_cache["last_results"] = res

    out = np.empty((B, T, D), dtype=np.float32)
    flagged = []
    for core in range(NCORES):
        b, j = core // 4, core % 4
        o = res.results[core]["out"]                 # [lev, t, d]
        v89 = res.results[core]["v89"]               # [lev, t, 2]
        for l in range(NLEV):
            r0 = P * (4 * l + j)
            out[b, r0:r0 + P, :] = o[l]
            gap = v89[l, :, 0].astype(np.float64) - v89[l, :, 1].astype(np.float64)
            for r in np.nonzero(gap < GAPTHR)[0]:
                flagged.append((b, r0 + int(r)))

    # near-tie rows: recompute exactly on host (selection ambiguous on device)
    for b, t in flagged:
        _fix_row(out[b], x[b], W_q, W_k, gain, bias, mix, scale, t)

    # degenerate t=0 rows: uniform attention over ALL positions
    for b in range(B):
        msg0 = x[b].sum(axis=0, dtype=np.float32) * np.float32(1.0 / T)
        blended = np.float32(mix) * x[b, 0] + np.float32(1.0 - mix) * msg0
        pre = blended * gain + bias
        out[b, 0, :] = (_gelu_exact_np(pre.astype(np.float64))
                        * scale).astype(np.float32)
    return out
